# revision 14
# baseline (speedup 1.0000x reference)
"""Trainium2 Bass kernel for LGRL classifier decoder (segment softmax-pool MLP).

Math (reference):
    extra = io_embed.reshape(B, Y)[segment_ids]                # (T, Y)
    h1 = relu([ps_data, extra] @ W1 + b1)
    h2 = relu(h1 @ W2 + b2)
    logits = (h2 @ W3 + b3)[:, 0]
    w = segment_softmax(logits)
    pooled = segment_sum(w * ps_data)                          # (B, X)
    out = relu(pooled @ Wf1 + bf1) @ Wf2 + bf2                 # (B, 2)

Key transformations used here:
  * [ps, extra] @ W1 = ps @ W1a + onehot(seg) @ (io_flat @ W1b + b1):
    the extra-part matmul collapses to a tiny (B, Y) @ (Y, H) precompute
    plus a rank-B broadcast matmul (one-hot), cutting PE work ~5x.
  * per-segment max subtraction in the softmax is dropped: softmax weights
    are invariant to any per-segment shift and logits are O(1) here, so
    exp() is safe in fp32.  b3 is dropped for the same reason (uniform
    logit shift cancels in the softmax).
  * segment sums are one-hot matmuls on the TensorEngine; per-core partial
    (num, den) are AllReduce'd across the 8 cores; final_fc is computed
    redundantly on every core.
  * matmuls run in bf16 (4x fp32 PE rate); accumulation is fp32 in PSUM.
    Validated absmax-relative error vs the fp32 reference: ~5e-3.

Sharding: the packed-token dim T is split evenly across the 8 cores; the
small MLP weights are replicated.  One-hot segment matrices are built on
the host (index prep) and shipped as bf16.
"""

import numpy as np
import ml_dtypes

import concourse.bass as bass
import concourse.mybir as mybir
import concourse.tile as tile
from concourse import bacc
from concourse.bass_utils import run_bass_kernel_spmd
from concourse.masks import make_identity

B = 64
T = 65536
X = 512
KIO = 5
Y = X * KIO          # 2560
H = 512
NCORES = 8
P = 128
FP32 = mybir.dt.float32
BF16 = mybir.dt.bfloat16
AF = mybir.ActivationFunctionType
ALU = mybir.AluOpType

KC = X // P          # 4 contraction chunks for 512-dims
HC = H // P          # 4 output chunks for 512-dims
NKB = Y // P         # 20 contraction chunks of W1b
MT = 512             # tokens per MLP tile
NSUB = MT // P       # 128-token subtiles per MLP tile


def build(tloc=T // NCORES):
    """Build + compile the SPMD kernel for per-core token count `tloc`."""
    nt = tloc // MT
    nc = bacc.Bacc(
        "TRN2", target_bir_lowering=False, debug=False, num_devices=NCORES
    )

    ps = nc.dram_tensor("ps", [tloc, X], FP32, kind="ExternalInput").ap()
    stm = nc.dram_tensor("stm", [tloc, B], BF16, kind="ExternalInput").ap()
    st = nc.dram_tensor("st", [B, tloc], BF16, kind="ExternalInput").ap()
    ioT = nc.dram_tensor("ioT", [Y + 1, B], FP32, kind="ExternalInput").ap()
    w1 = nc.dram_tensor("w1", [X + Y, H], FP32, kind="ExternalInput").ap()
    b1 = nc.dram_tensor("b1", [H], FP32, kind="ExternalInput").ap()
    w2 = nc.dram_tensor("w2", [H, H], FP32, kind="ExternalInput").ap()
    b2 = nc.dram_tensor("b2", [H], FP32, kind="ExternalInput").ap()
    w3 = nc.dram_tensor("w3", [H, 1], FP32, kind="ExternalInput").ap()
    wf1 = nc.dram_tensor("wf1", [H, H], FP32, kind="ExternalInput").ap()
    bf1_t = nc.dram_tensor("bf1", [H], FP32, kind="ExternalInput").ap()
    wf2 = nc.dram_tensor("wf2", [H, 2], FP32, kind="ExternalInput").ap()
    bf2_t = nc.dram_tensor("bf2", [2], FP32, kind="ExternalInput").ap()
    outT = nc.dram_tensor("outT", [2, B], FP32, kind="ExternalOutput").ap()

    with tile.TileContext(nc) as tc:
        with (
            tc.tile_pool(name="const", bufs=1) as cpool,
            tc.tile_pool(name="work", bufs=2) as wpool,
            tc.tile_pool(name="psum", bufs=1, space="PSUM") as ppool,
            tc.tile_pool(name="dram", bufs=1, space="DRAM") as dpool,
        ):
            # ---------------- constants ----------------
            ident = cpool.tile([P, P], BF16)
            make_identity(nc, ident)
            identf = cpool.tile([1, 1], FP32)
            nc.gpsimd.memset(identf, 1.0)

            w1a_sb = cpool.tile([P, KC * H], BF16)
            w2_sb = cpool.tile([P, KC * H], BF16)
            wf1_sb = cpool.tile([P, KC * H], BF16)
            for kc in range(KC):
                nc.gpsimd.dma_start(
                    w1a_sb[:, kc * H : (kc + 1) * H], w1[kc * P : (kc + 1) * P, :]
                )
                nc.gpsimd.dma_start(
                    w2_sb[:, kc * H : (kc + 1) * H], w2[kc * P : (kc + 1) * P, :]
                )
                nc.gpsimd.dma_start(
                    wf1_sb[:, kc * H : (kc + 1) * H], wf1[kc * P : (kc + 1) * P, :]
                )
            w1b_sb = cpool.tile([P, (NKB + 1) * H], BF16)
            ioT_sb = cpool.tile([P, (NKB + 1) * B], BF16)
            for kb in range(NKB):
                nc.gpsimd.dma_start(
                    w1b_sb[:, kb * H : (kb + 1) * H],
                    w1[X + kb * P : X + (kb + 1) * P, :],
                )
                nc.gpsimd.dma_start(
                    ioT_sb[:, kb * B : (kb + 1) * B], ioT[kb * P : (kb + 1) * P, :]
                )
            nc.gpsimd.dma_start(w1b_sb[0:1, NKB * H : NKB * H + H], b1[None, :])
            nc.gpsimd.dma_start(ioT_sb[0:1, NKB * B : NKB * B + B], ioT[Y : Y + 1, :])

            w3_sb = cpool.tile([P, KC], BF16)
            wf2_sb = cpool.tile([P, HC * 2], BF16)
            for kc in range(KC):
                nc.gpsimd.dma_start(w3_sb[:, kc : kc + 1], w3[kc * P : (kc + 1) * P, :])
                nc.gpsimd.dma_start(
                    wf2_sb[:, kc * 2 : (kc + 1) * 2], wf2[kc * P : (kc + 1) * P, :]
                )
            b2_sb = cpool.tile([P, HC], FP32)
            nc.sync.dma_start(b2_sb, b2.rearrange("(c p) -> p c", p=P))
            bf1_sb = cpool.tile([P, HC], FP32)
            nc.sync.dma_start(bf1_sb, bf1_t.rearrange("(c p) -> p c", p=P))
            bf2_sb = cpool.tile([2, 1], FP32)
            nc.sync.dma_start(bf2_sb, bf2_t[:, None])

            st_sb = cpool.tile([B, tloc], BF16)
            nc.sync.dma_start(st_sb, st)
            stm_sb = cpool.tile([P, tloc // P, B], BF16)
            nc.sync.dma_start(stm_sb, stm.rearrange("(n p) b -> p n b", p=P))

            # ---------------- seg_contrib = io_flat @ W1b + b1  (B, H) ----------------
            seg_psum = ppool.tile([P, H], FP32, tag="h1h2", bufs=2)
            for kb in range(NKB + 1):
                kd = P if kb < NKB else 1
                nc.tensor.matmul(
                    seg_psum[0:B, :],
                    ioT_sb[0:kd, kb * B : (kb + 1) * B],
                    w1b_sb[0:kd, kb * H : (kb + 1) * H],
                    start=(kb == 0),
                    stop=(kb == NKB),
                )
            seg_sb = cpool.tile([B, H], BF16)
            nc.vector.tensor_copy(seg_sb, seg_psum[0:B, :])

            # ---------------- main loop over MLP tiles ----------------
            pool_psum = ppool.tile([P, H], FP32, tag="pool", bufs=1)
            den_psum = ppool.tile([B, 1], FP32, tag="den", bufs=1)
            prev = None  # (ps_bf, e_col) of previous tile, pooled late

            def emit_pool(j, ps_bf, e_col, e_colb):
                ps_sc = wpool.tile([P, NSUB, X], BF16, tag="psc", bufs=2)
                for s in range(NSUB):
                    nc.vector.tensor_scalar_mul(
                        ps_sc[:, s, :], ps_bf[:, s, :], e_col[:, s : s + 1]
                    )
                    sub = j * NSUB + s
                    first = sub == 0
                    last = sub == nt * NSUB - 1
                    nc.tensor.matmul(
                        pool_psum[0:B, :],
                        stm_sb[:, sub, :],
                        ps_sc[:, s, :],
                        start=first,
                        stop=last,
                    )
                    nc.tensor.matmul(
                        den_psum[:, 0:1],
                        stm_sb[:, sub, :],
                        e_colb[:, s : s + 1],
                        start=first,
                        stop=last,
                    )

            for j in range(nt):
                ps_bf = wpool.tile([P, NSUB, X], BF16, tag="ps", bufs=3)
                nc.gpsimd.dma_start(
                    ps_bf, ps.rearrange("(j s p) f -> j p s f", p=P, s=NSUB)[j]
                )
                # transpose ps tile to feature-major (bf16, via PE)
                psT_sb = wpool.tile([P, KC, MT], BF16, tag="psT", bufs=2)
                for kc in range(KC):
                    tp = ppool.tile([P, MT], BF16, tag="psTp", bufs=2)
                    for s in range(NSUB):
                        nc.tensor.transpose(
                            tp[:, s * P : (s + 1) * P],
                            ps_bf[:, s, kc * P : (kc + 1) * P],
                            ident,
                        )
                    if kc % 2 == 0:
                        nc.vector.tensor_copy(psT_sb[:, kc, :], tp)
                    else:
                        nc.scalar.activation(psT_sb[:, kc, :], tp, AF.Copy)

                # previous tile's e-transposes (PE) early, pooling later
                if prev is not None:
                    pj, p_psbf, p_erow = prev
                    eTp = ppool.tile([P, NSUB], FP32, tag="leT", bufs=2)
                    for s in range(NSUB):
                        nc.tensor.transpose(
                            eTp[:, s : s + 1],
                            p_erow[0:1, s * P : (s + 1) * P],
                            identf[0:1, 0:1],
                        )
                    e_col = wpool.tile([P, NSUB], FP32, tag="ecol", bufs=2)
                    nc.vector.tensor_copy(e_col, eTp)
                    e_colb = wpool.tile([P, NSUB], BF16, tag="ecolb", bufs=2)
                    nc.vector.tensor_copy(e_colb, eTp)

                # h1 = relu(psT.T-major matmuls + seg broadcast)
                h1_sb = wpool.tile([P, KC, MT], BF16, tag="h1", bufs=2)
                for hc in range(HC):
                    h1p = ppool.tile([P, MT], FP32, tag="h1h2", bufs=2)
                    for kc in range(KC):
                        nc.tensor.matmul(
                            h1p,
                            w1a_sb[:, kc * H + hc * P : kc * H + (hc + 1) * P],
                            psT_sb[:, kc, :],
                            start=(kc == 0),
                            stop=False,
                        )
                    nc.tensor.matmul(
                        h1p,
                        seg_sb[:, hc * P : (hc + 1) * P],
                        st_sb[:, j * MT : (j + 1) * MT],
                        start=False,
                        stop=True,
                    )
                    if hc % 2 == 0:
                        nc.scalar.activation(h1_sb[:, hc, :], h1p, AF.Relu)
                    else:
                        nc.vector.tensor_scalar_max(h1_sb[:, hc, :], h1p, 0.0)

                # previous tile's pooling (its DVE scale ran during our h1)
                if prev is not None:
                    emit_pool(prev[0], prev[1], e_col, e_colb)
                    prev = None

                # h2
                h2_sb = wpool.tile([P, KC, MT], BF16, tag="h2", bufs=2)
                for hc in range(HC):
                    h2p = ppool.tile([P, MT], FP32, tag="h1h2", bufs=2)
                    for kc in range(KC):
                        nc.tensor.matmul(
                            h2p,
                            w2_sb[:, kc * H + hc * P : kc * H + (hc + 1) * P],
                            h1_sb[:, kc, :],
                            start=(kc == 0),
                            stop=(kc == KC - 1),
                        )
                    if hc % 2 == 0:
                        nc.scalar.activation(
                            h2_sb[:, hc, :], h2p, AF.Relu, bias=b2_sb[:, hc : hc + 1]
                        )
                    else:
                        nc.vector.tensor_scalar(
                            h2_sb[:, hc, :],
                            h2p,
                            b2_sb[:, hc : hc + 1],
                            0.0,
                            op0=ALU.add,
                            op1=ALU.max,
                        )

                # logits -> e = exp(logits)   (b3 dropped: cancels in softmax)
                lp = ppool.tile([1, MT], FP32, tag="leT", bufs=2)
                for kc in range(KC):
                    nc.tensor.matmul(
                        lp,
                        w3_sb[:, kc : kc + 1],
                        h2_sb[:, kc, :],
                        start=(kc == 0),
                        stop=(kc == KC - 1),
                    )
                e_row = wpool.tile([1, MT], FP32, tag="erow", bufs=2)
                nc.scalar.activation(e_row, lp, AF.Exp)

                prev = (j, ps_bf, e_row)

            # last tile's e-transpose + pooling
            pj, p_psbf, p_erow = prev
            eTp = ppool.tile([P, NSUB], FP32, tag="leT", bufs=2)
            for s in range(NSUB):
                nc.tensor.transpose(
                    eTp[:, s : s + 1],
                    p_erow[0:1, s * P : (s + 1) * P],
                    identf[0:1, 0:1],
                )
            e_col = wpool.tile([P, NSUB], FP32, tag="ecol", bufs=2)
            nc.vector.tensor_copy(e_col, eTp)
            e_colb = wpool.tile([P, NSUB], BF16, tag="ecolb", bufs=2)
            nc.vector.tensor_copy(e_colb, eTp)
            emit_pool(pj, p_psbf, e_col, e_colb)

            # ---------------- combine across cores ----------------
            num_sb = wpool.tile([B, H], FP32, tag="fin_num", bufs=1)
            nc.vector.tensor_copy(num_sb, pool_psum[0:B, :])
            den_sb = wpool.tile([B, 1], FP32, tag="fin_den", bufs=1)
            nc.vector.tensor_copy(den_sb, den_psum[:, 0:1])

            cc_in = dpool.tile([B, H + 1], FP32)
            cc_out = dpool.tile([B, H + 1], FP32)
            nc.sync.dma_start(cc_in[:, 0:H], num_sb)
            nc.sync.dma_start(cc_in[:, H : H + 1], den_sb)
            nc.gpsimd.collective_compute(
                "AllReduce",
                ALU.add,
                replica_groups=[list(range(NCORES))],
                ins=[cc_in.opt()],
                outs=[cc_out.opt()],
            )
            numg = wpool.tile([B, H], FP32, tag="fin_numg", bufs=1)
            deng = wpool.tile([B, 1], FP32, tag="fin_deng", bufs=1)
            nc.sync.dma_start(numg, cc_out[:, 0:H])
            nc.sync.dma_start(deng, cc_out[:, H : H + 1])

            rec = wpool.tile([B, 1], FP32, tag="fin_rec", bufs=1)
            nc.vector.reciprocal(rec, deng)
            pooled = wpool.tile([B, H], BF16, tag="fin_pool", bufs=1)
            nc.vector.tensor_scalar_mul(pooled, numg, rec[:, 0:1])

            # final_fc (redundant on every core)
            ptp = ppool.tile([P, KC * B], BF16, tag="psTp", bufs=2)
            for kc in range(KC):
                nc.tensor.transpose(
                    ptp[:, kc * B : (kc + 1) * B],
                    pooled[:, kc * P : (kc + 1) * P],
                    ident[0:B, 0:B],
                )
            pooledT = wpool.tile([P, KC * B], BF16, tag="fin_poolT", bufs=1)
            nc.vector.tensor_copy(pooledT, ptp)

            hf_sb = wpool.tile([P, HC * B], BF16, tag="fin_hf", bufs=1)
            for hc in range(HC):
                hfp = ppool.tile([P, B], FP32, tag="h1h2", bufs=2)
                for kc in range(KC):
                    nc.tensor.matmul(
                        hfp,
                        wf1_sb[:, kc * H + hc * P : kc * H + (hc + 1) * P],
                        pooledT[:, kc * B : (kc + 1) * B],
                        start=(kc == 0),
                        stop=(kc == KC - 1),
                    )
                nc.scalar.activation(
                    hf_sb[:, hc * B : (hc + 1) * B],
                    hfp,
                    AF.Relu,
                    bias=bf1_sb[:, hc : hc + 1],
                )
            op = ppool.tile([2, B], FP32, tag="leT", bufs=2)
            for hc in range(HC):
                nc.tensor.matmul(
                    op,
                    wf2_sb[:, hc * 2 : (hc + 1) * 2],
                    hf_sb[:, hc * B : (hc + 1) * B],
                    start=(hc == 0),
                    stop=(hc == HC - 1),
                )
            o_sb = wpool.tile([2, B], FP32, tag="fin_o", bufs=1)
            nc.vector.tensor_scalar_add(o_sb, op, bf2_sb[:, 0:1])
            nc.sync.dma_start(outT, o_sb)

    nc.compile()
    return nc


def prep_in_maps(inputs, tloc=T // NCORES, ncores=NCORES):
    """Shard the full inputs into per-core input maps (host-side prep only:
    slicing, transposes of small tensors, one-hot index materialization)."""
    bf = ml_dtypes.bfloat16
    ps = np.ascontiguousarray(np.asarray(inputs["ps_data"], np.float32))
    sid = np.asarray(inputs["segment_ids"], np.int64)
    io_flat = np.asarray(inputs["io_embed"], np.float32).reshape(B, -1)
    ttot = tloc * ncores
    assert ps.shape[0] == ttot and sid.shape[0] == ttot

    onehot = np.zeros((ttot, B), bf)
    onehot[np.arange(ttot), sid] = 1
    onehotT = np.ascontiguousarray(onehot.T)

    ioT = np.concatenate(
        [io_flat.T, np.ones((1, B), np.float32)], axis=0
    ).astype(np.float32)

    shared = {
        "ioT": ioT,
        "w1": np.asarray(inputs["W1"], np.float32),
        "b1": np.asarray(inputs["b1"], np.float32),
        "w2": np.asarray(inputs["W2"], np.float32),
        "b2": np.asarray(inputs["b2"], np.float32),
        "w3": np.asarray(inputs["W3"], np.float32),
        "wf1": np.asarray(inputs["Wf1"], np.float32),
        "bf1": np.asarray(inputs["bf1"], np.float32),
        "wf2": np.asarray(inputs["Wf2"], np.float32),
        "bf2": np.asarray(inputs["bf2"], np.float32),
    }
    in_maps = []
    for c in range(ncores):
        lo, hi = c * tloc, (c + 1) * tloc
        in_maps.append(
            {
                "ps": ps[lo:hi],
                "stm": np.ascontiguousarray(onehot[lo:hi]),
                "st": np.ascontiguousarray(onehotT[:, lo:hi]),
                **shared,
            }
        )
    return in_maps


_NC_CACHE = {}


def _get_nc(tloc):
    if tloc not in _NC_CACHE:
        _NC_CACHE[tloc] = build(tloc)
    return _NC_CACHE[tloc]


def run(inputs, trace=False):
    nc = _get_nc(T // NCORES)
    in_maps = prep_in_maps(inputs)
    res = run_bass_kernel_spmd(nc, in_maps, core_ids=list(range(NCORES)), trace=trace)
    out = np.ascontiguousarray(res.results[0]["outT"].T).astype(np.float32)
    return out, res


def kernel(**inputs):
    out, _ = run(inputs)
    return out


# revision 21
# speedup vs baseline: 1.0536x; 1.0536x over previous
"""Trainium2 Bass kernel for LGRL classifier decoder (segment softmax-pool MLP).

Math (reference):
    extra = io_embed.reshape(B, Y)[segment_ids]                # (T, Y)
    h1 = relu([ps_data, extra] @ W1 + b1)
    h2 = relu(h1 @ W2 + b2)
    logits = (h2 @ W3 + b3)[:, 0]
    w = segment_softmax(logits)
    pooled = segment_sum(w * ps_data)                          # (B, X)
    out = relu(pooled @ Wf1 + bf1) @ Wf2 + bf2                 # (B, 2)

Key transformations used here:
  * [ps, extra] @ W1 = ps @ W1a + onehot(seg) @ (io_flat @ W1b + b1):
    the extra-part matmul collapses to a tiny (B, Y) @ (Y, H) precompute
    plus a rank-B broadcast matmul (one-hot), cutting PE work ~5x.
  * per-segment max subtraction in the softmax is dropped: softmax weights
    are invariant to any per-segment shift and logits are O(1) here, so
    exp() is safe in fp32.  b3 is dropped for the same reason (uniform
    logit shift cancels in the softmax).
  * segment sums are one-hot matmuls on the TensorEngine; per-core partial
    (num, den) are AllReduce'd across the 8 cores; final_fc is computed
    redundantly on every core.
  * matmuls run in bf16 (4x fp32 PE rate); accumulation is fp32 in PSUM.
    Validated absmax-relative error vs the fp32 reference: ~5e-3.

Sharding: the packed-token dim T is split evenly across the 8 cores; the
small MLP weights are replicated.  One-hot segment matrices are built on
the host (index prep) and shipped as bf16.
"""

import numpy as np
import ml_dtypes

import concourse.bass as bass
import concourse.mybir as mybir
import concourse.tile as tile
from concourse import bacc
from concourse.bass_utils import run_bass_kernel_spmd
from concourse.masks import make_identity

B = 64
T = 65536
X = 512
KIO = 5
Y = X * KIO          # 2560
H = 512
NCORES = 8
P = 128
FP32 = mybir.dt.float32
BF16 = mybir.dt.bfloat16
AF = mybir.ActivationFunctionType
ALU = mybir.AluOpType

KC = X // P          # 4 contraction chunks for 512-dims
HC = H // P          # 4 output chunks for 512-dims
NKB = Y // P         # 20 contraction chunks of W1b
MT = 512             # tokens per MLP tile
NSUB = MT // P       # 128-token subtiles per MLP tile


def build(tloc=T // NCORES):
    """Build + compile the SPMD kernel for per-core token count `tloc`."""
    nt = tloc // MT
    BR = B // NCORES  # segment rows finalized per core
    nc = bacc.Bacc(
        "TRN2", target_bir_lowering=False, debug=False, num_devices=NCORES
    )

    ps = nc.dram_tensor("ps", [tloc, X], FP32, kind="ExternalInput").ap()
    stm = nc.dram_tensor("stm", [tloc, B], BF16, kind="ExternalInput").ap()
    st = nc.dram_tensor("st", [B, tloc], BF16, kind="ExternalInput").ap()
    ioT = nc.dram_tensor("ioT", [Y + 1, B], FP32, kind="ExternalInput").ap()
    w1 = nc.dram_tensor("w1", [X + Y, H], FP32, kind="ExternalInput").ap()
    b1 = nc.dram_tensor("b1", [H], FP32, kind="ExternalInput").ap()
    w2 = nc.dram_tensor("w2", [H, H], FP32, kind="ExternalInput").ap()
    b2 = nc.dram_tensor("b2", [H], FP32, kind="ExternalInput").ap()
    w3 = nc.dram_tensor("w3", [H, 1], FP32, kind="ExternalInput").ap()
    wf1 = nc.dram_tensor("wf1", [H, H], FP32, kind="ExternalInput").ap()
    bf1_t = nc.dram_tensor("bf1", [H], FP32, kind="ExternalInput").ap()
    wf2 = nc.dram_tensor("wf2", [H, 2], FP32, kind="ExternalInput").ap()
    bf2_t = nc.dram_tensor("bf2", [2], FP32, kind="ExternalInput").ap()
    outT = nc.dram_tensor("outT", [2, B // NCORES], FP32, kind="ExternalOutput").ap()

    with tile.TileContext(nc) as tc:
        with (
            tc.tile_pool(name="const", bufs=1) as cpool,
            tc.tile_pool(name="work", bufs=2) as wpool,
            tc.tile_pool(name="psum", bufs=1, space="PSUM") as ppool,
            tc.tile_pool(name="dram", bufs=1, space="DRAM") as dpool,
        ):
            # ---------------- constants ----------------
            ident = cpool.tile([P, P], BF16)
            make_identity(nc, ident)
            identf = cpool.tile([1, 1], FP32)
            nc.gpsimd.memset(identf, 1.0)

            # prefetch the first ps tiles before the bulky weight DMAs so the
            # PE transposes can start immediately
            NPRE = min(3, nt)
            pre_ps = []
            for j in range(NPRE):
                ps_bf = wpool.tile([P, NSUB, X], BF16, tag="ps", bufs=4, name=f"ps_bf_{j}")
                nc.gpsimd.dma_start(
                    ps_bf, ps.rearrange("(j s p) f -> j p s f", p=P, s=NSUB)[j]
                )
                pre_ps.append(ps_bf)

            # warm up the collective path early (rendezvous/setup overlaps the
            # main loop); the result is copied into an SBUF tile that the final
            # output add consumes with weight 0 so it cannot be DCE'd.
            wm_sb = cpool.tile([2, BR], FP32)
            nc.gpsimd.memset(wm_sb, 0.0)
            wm_in = dpool.tile([NCORES * 2, BR], FP32)
            wm_out = dpool.tile([2, BR], FP32)
            for c in range(NCORES):
                nc.sync.dma_start(wm_in[c * 2 : (c + 1) * 2, :], wm_sb)
            nc.gpsimd.collective_compute(
                "ReduceScatter",
                ALU.add,
                replica_groups=[list(range(NCORES))],
                ins=[wm_in.opt()],
                outs=[wm_out.opt()],
            )
            wz_sb = cpool.tile([2, BR], FP32)
            nc.sync.dma_start(wz_sb, wm_out)

            w1a_sb = cpool.tile([P, KC * H], BF16)
            w2_sb = cpool.tile([P, KC * H], BF16)
            wf1_sb = cpool.tile([P, KC * H], BF16)
            for kc in range(KC):
                nc.gpsimd.dma_start(
                    w1a_sb[:, kc * H : (kc + 1) * H], w1[kc * P : (kc + 1) * P, :]
                )
                nc.gpsimd.dma_start(
                    w2_sb[:, kc * H : (kc + 1) * H], w2[kc * P : (kc + 1) * P, :]
                )
                nc.gpsimd.dma_start(
                    wf1_sb[:, kc * H : (kc + 1) * H], wf1[kc * P : (kc + 1) * P, :]
                )
            w1b_sb = cpool.tile([P, (NKB + 1) * H], BF16)
            ioT_sb = cpool.tile([P, (NKB + 1) * B], BF16)
            for kb in range(NKB):
                nc.gpsimd.dma_start(
                    w1b_sb[:, kb * H : (kb + 1) * H],
                    w1[X + kb * P : X + (kb + 1) * P, :],
                )
                nc.gpsimd.dma_start(
                    ioT_sb[:, kb * B : (kb + 1) * B], ioT[kb * P : (kb + 1) * P, :]
                )
            nc.gpsimd.dma_start(w1b_sb[0:1, NKB * H : NKB * H + H], b1[None, :])
            nc.gpsimd.dma_start(ioT_sb[0:1, NKB * B : NKB * B + B], ioT[Y : Y + 1, :])

            w3_sb = cpool.tile([P, KC], BF16)
            wf2_sb = cpool.tile([P, HC * 2], BF16)
            for kc in range(KC):
                nc.gpsimd.dma_start(w3_sb[:, kc : kc + 1], w3[kc * P : (kc + 1) * P, :])
                nc.gpsimd.dma_start(
                    wf2_sb[:, kc * 2 : (kc + 1) * 2], wf2[kc * P : (kc + 1) * P, :]
                )
            b2_sb = cpool.tile([P, HC], FP32)
            nc.sync.dma_start(b2_sb, b2.rearrange("(c p) -> p c", p=P))
            bf1_sb = cpool.tile([P, HC], FP32)
            nc.sync.dma_start(bf1_sb, bf1_t.rearrange("(c p) -> p c", p=P))
            bf2_sb = cpool.tile([2, 1], FP32)
            nc.sync.dma_start(bf2_sb, bf2_t[:, None])

            st_sb = cpool.tile([B, tloc], BF16)
            nc.sync.dma_start(st_sb, st)
            stm_sb = cpool.tile([P, tloc // P, B], BF16)
            nc.sync.dma_start(stm_sb, stm.rearrange("(n p) b -> p n b", p=P))

            # ---------------- seg_contrib = io_flat @ W1b + b1  (B, H) ----------------
            seg_psum = ppool.tile([P, H], FP32, tag="h1h2", bufs=2)
            for kb in range(NKB + 1):
                kd = P if kb < NKB else 1
                nc.tensor.matmul(
                    seg_psum[0:B, :],
                    ioT_sb[0:kd, kb * B : (kb + 1) * B],
                    w1b_sb[0:kd, kb * H : (kb + 1) * H],
                    start=(kb == 0),
                    stop=(kb == NKB),
                )
            seg_sb = cpool.tile([B, H], BF16)
            nc.vector.tensor_copy(seg_sb, seg_psum[0:B, :])

            # ---------------- main loop over MLP tiles ----------------
            pool_psum = ppool.tile([P, H], FP32, tag="pool", bufs=1)
            den_psum = ppool.tile([B, 1], FP32, tag="den", bufs=1)
            prev = None  # (ps_bf, e_col) of previous tile, pooled late

            def emit_pool(j, ps_bf, e_col, e_colb):
                ps_sc = wpool.tile([P, NSUB, X], BF16, tag="psc", bufs=2)
                for s in range(NSUB):
                    nc.vector.tensor_scalar_mul(
                        ps_sc[:, s, :], ps_bf[:, s, :], e_col[:, s : s + 1]
                    )
                    sub = j * NSUB + s
                    first = sub == 0
                    last = sub == nt * NSUB - 1
                    nc.tensor.matmul(
                        pool_psum[0:B, :],
                        stm_sb[:, sub, :],
                        ps_sc[:, s, :],
                        start=first,
                        stop=last,
                    )
                    nc.tensor.matmul(
                        den_psum[:, 0:1],
                        stm_sb[:, sub, :],
                        e_colb[:, s : s + 1],
                        start=first,
                        stop=last,
                    )

            for j in range(nt):
                if j < NPRE:
                    ps_bf = pre_ps[j]
                else:
                    ps_bf = wpool.tile([P, NSUB, X], BF16, tag="ps", bufs=4)
                    nc.gpsimd.dma_start(
                        ps_bf, ps.rearrange("(j s p) f -> j p s f", p=P, s=NSUB)[j]
                    )
                # transpose ps tile to feature-major (bf16, via PE)
                psT_sb = wpool.tile([P, KC, MT], BF16, tag="psT", bufs=2)
                for kc in range(KC):
                    tp = ppool.tile([P, MT], BF16, tag="psTp", bufs=2)
                    for s in range(NSUB):
                        nc.tensor.transpose(
                            tp[:, s * P : (s + 1) * P],
                            ps_bf[:, s, kc * P : (kc + 1) * P],
                            ident,
                        )
                    if kc % 2 == 0:
                        nc.vector.tensor_copy(psT_sb[:, kc, :], tp)
                    else:
                        nc.scalar.activation(psT_sb[:, kc, :], tp, AF.Copy)

                # previous tile's e-transposes (PE) early, pooling later
                if prev is not None:
                    pj, p_psbf, p_erow = prev
                    eTp = ppool.tile([P, NSUB], FP32, tag="leT", bufs=2)
                    for s in range(NSUB):
                        nc.tensor.transpose(
                            eTp[:, s : s + 1],
                            p_erow[0:1, s * P : (s + 1) * P],
                            identf[0:1, 0:1],
                        )
                    e_col = wpool.tile([P, NSUB], FP32, tag="ecol", bufs=2)
                    nc.vector.tensor_copy(e_col, eTp)
                    e_colb = wpool.tile([P, NSUB], BF16, tag="ecolb", bufs=2)
                    nc.vector.tensor_copy(e_colb, eTp)

                # h1 = relu(psT.T-major matmuls + seg broadcast)
                h1_sb = wpool.tile([P, KC, MT], BF16, tag="h1", bufs=2)
                for hc in range(HC):
                    h1p = ppool.tile([P, MT], FP32, tag="h1h2", bufs=2)
                    for kc in range(KC):
                        nc.tensor.matmul(
                            h1p,
                            w1a_sb[:, kc * H + hc * P : kc * H + (hc + 1) * P],
                            psT_sb[:, kc, :],
                            start=(kc == 0),
                            stop=False,
                        )
                    nc.tensor.matmul(
                        h1p,
                        seg_sb[:, hc * P : (hc + 1) * P],
                        st_sb[:, j * MT : (j + 1) * MT],
                        start=False,
                        stop=True,
                    )
                    if hc % 2 == 0:
                        nc.scalar.activation(h1_sb[:, hc, :], h1p, AF.Relu)
                    else:
                        nc.vector.tensor_scalar_max(h1_sb[:, hc, :], h1p, 0.0)

                # previous tile's pooling (its DVE scale ran during our h1)
                if prev is not None:
                    emit_pool(prev[0], prev[1], e_col, e_colb)
                    prev = None

                # h2
                h2_sb = wpool.tile([P, KC, MT], BF16, tag="h2", bufs=2)
                for hc in range(HC):
                    h2p = ppool.tile([P, MT], FP32, tag="h1h2", bufs=2)
                    for kc in range(KC):
                        nc.tensor.matmul(
                            h2p,
                            w2_sb[:, kc * H + hc * P : kc * H + (hc + 1) * P],
                            h1_sb[:, kc, :],
                            start=(kc == 0),
                            stop=(kc == KC - 1),
                        )
                    if hc % 2 == 0:
                        nc.scalar.activation(
                            h2_sb[:, hc, :], h2p, AF.Relu, bias=b2_sb[:, hc : hc + 1]
                        )
                    else:
                        nc.vector.tensor_scalar(
                            h2_sb[:, hc, :],
                            h2p,
                            b2_sb[:, hc : hc + 1],
                            0.0,
                            op0=ALU.add,
                            op1=ALU.max,
                        )

                # logits -> e = exp(logits)   (b3 dropped: cancels in softmax)
                lp = ppool.tile([1, MT], FP32, tag="leT", bufs=2)
                for kc in range(KC):
                    nc.tensor.matmul(
                        lp,
                        w3_sb[:, kc : kc + 1],
                        h2_sb[:, kc, :],
                        start=(kc == 0),
                        stop=(kc == KC - 1),
                    )
                e_row = wpool.tile([1, MT], FP32, tag="erow", bufs=2)
                nc.scalar.activation(e_row, lp, AF.Exp)

                prev = (j, ps_bf, e_row)

            # last tile's e-transpose + pooling
            pj, p_psbf, p_erow = prev
            eTp = ppool.tile([P, NSUB], FP32, tag="leT", bufs=2)
            for s in range(NSUB):
                nc.tensor.transpose(
                    eTp[:, s : s + 1],
                    p_erow[0:1, s * P : (s + 1) * P],
                    identf[0:1, 0:1],
                )
            e_col = wpool.tile([P, NSUB], FP32, tag="ecol", bufs=2)
            nc.vector.tensor_copy(e_col, eTp)
            e_colb = wpool.tile([P, NSUB], BF16, tag="ecolb", bufs=2)
            nc.vector.tensor_copy(e_colb, eTp)
            emit_pool(pj, p_psbf, e_col, e_colb)

            # ---------------- combine across cores ----------------
            # ReduceScatter the (num | den) partials: core c receives the
            # fully-reduced rows for segments [c*BR, (c+1)*BR) and finalizes
            # only those; the host concatenates the 8 per-core outputs.
            num_sb = wpool.tile([B, H], FP32, tag="fin_num", bufs=1)
            nc.vector.tensor_copy(num_sb, pool_psum[0:B, :])
            den_sb = wpool.tile([B, 1], FP32, tag="fin_den", bufs=1)
            nc.vector.tensor_copy(den_sb, den_psum[:, 0:1])

            cc_in = dpool.tile([B, H + 1], FP32)
            cc_out = dpool.tile([BR, H + 1], FP32)
            nc.sync.dma_start(cc_in[:, 0:H], num_sb)
            nc.sync.dma_start(cc_in[:, H : H + 1], den_sb)
            nc.gpsimd.collective_compute(
                "ReduceScatter",
                ALU.add,
                replica_groups=[list(range(NCORES))],
                ins=[cc_in.opt()],
                outs=[cc_out.opt()],
            )
            numg = wpool.tile([BR, H], FP32, tag="fin_numg", bufs=1)
            deng = wpool.tile([BR, 1], FP32, tag="fin_deng", bufs=1)
            nc.sync.dma_start(numg, cc_out[:, 0:H])
            nc.sync.dma_start(deng, cc_out[:, H : H + 1])

            rec = wpool.tile([BR, 1], FP32, tag="fin_rec", bufs=1)
            nc.vector.reciprocal(rec, deng)
            pooled = wpool.tile([BR, H], BF16, tag="fin_pool", bufs=1)
            nc.vector.tensor_scalar_mul(pooled, numg, rec[:, 0:1])

            # final_fc on this core's BR segment rows
            ptp = ppool.tile([P, KC * BR], BF16, tag="psTp", bufs=2)
            for kc in range(KC):
                nc.tensor.transpose(
                    ptp[:, kc * BR : (kc + 1) * BR],
                    pooled[:, kc * P : (kc + 1) * P],
                    ident[0:BR, 0:BR],
                )
            pooledT = wpool.tile([P, KC * BR], BF16, tag="fin_poolT", bufs=1)
            nc.vector.tensor_copy(pooledT, ptp)

            hf_sb = wpool.tile([P, HC * BR], BF16, tag="fin_hf", bufs=1)
            for hc in range(HC):
                hfp = ppool.tile([P, BR], FP32, tag="h1h2", bufs=2)
                for kc in range(KC):
                    nc.tensor.matmul(
                        hfp,
                        wf1_sb[:, kc * H + hc * P : kc * H + (hc + 1) * P],
                        pooledT[:, kc * BR : (kc + 1) * BR],
                        start=(kc == 0),
                        stop=(kc == KC - 1),
                    )
                nc.scalar.activation(
                    hf_sb[:, hc * BR : (hc + 1) * BR],
                    hfp,
                    AF.Relu,
                    bias=bf1_sb[:, hc : hc + 1],
                )
            op = ppool.tile([2, BR], FP32, tag="leT", bufs=2)
            for hc in range(HC):
                nc.tensor.matmul(
                    op,
                    wf2_sb[:, hc * 2 : (hc + 1) * 2],
                    hf_sb[:, hc * BR : (hc + 1) * BR],
                    start=(hc == 0),
                    stop=(hc == HC - 1),
                )
            o_sb = wpool.tile([2, BR], FP32, tag="fin_o", bufs=1)
            nc.vector.tensor_scalar_add(o_sb, op, bf2_sb[:, 0:1])
            # + zeros from the warmup collective (keeps it live; exact no-op)
            o2_sb = wpool.tile([2, BR], FP32, tag="fin_o2", bufs=1)
            nc.vector.tensor_add(o2_sb, o_sb, wz_sb)
            nc.sync.dma_start(outT, o2_sb)

    nc.compile()
    return nc


def prep_in_maps(inputs, tloc=T // NCORES, ncores=NCORES):
    """Shard the full inputs into per-core input maps (host-side prep only:
    slicing, transposes of small tensors, one-hot index materialization)."""
    bf = ml_dtypes.bfloat16
    ps = np.ascontiguousarray(np.asarray(inputs["ps_data"], np.float32))
    sid = np.asarray(inputs["segment_ids"], np.int64)
    io_flat = np.asarray(inputs["io_embed"], np.float32).reshape(B, -1)
    ttot = tloc * ncores
    assert ps.shape[0] == ttot and sid.shape[0] == ttot

    onehot = np.zeros((ttot, B), bf)
    onehot[np.arange(ttot), sid] = 1
    onehotT = np.ascontiguousarray(onehot.T)

    ioT = np.concatenate(
        [io_flat.T, np.ones((1, B), np.float32)], axis=0
    ).astype(np.float32)

    shared = {
        "ioT": ioT,
        "w1": np.asarray(inputs["W1"], np.float32),
        "b1": np.asarray(inputs["b1"], np.float32),
        "w2": np.asarray(inputs["W2"], np.float32),
        "b2": np.asarray(inputs["b2"], np.float32),
        "w3": np.asarray(inputs["W3"], np.float32),
        "wf1": np.asarray(inputs["Wf1"], np.float32),
        "bf1": np.asarray(inputs["bf1"], np.float32),
        "wf2": np.asarray(inputs["Wf2"], np.float32),
        "bf2": np.asarray(inputs["bf2"], np.float32),
    }
    in_maps = []
    for c in range(ncores):
        lo, hi = c * tloc, (c + 1) * tloc
        in_maps.append(
            {
                "ps": ps[lo:hi],
                "stm": np.ascontiguousarray(onehot[lo:hi]),
                "st": np.ascontiguousarray(onehotT[:, lo:hi]),
                **shared,
            }
        )
    return in_maps


_NC_CACHE = {}


def _get_nc(tloc):
    if tloc not in _NC_CACHE:
        _NC_CACHE[tloc] = build(tloc)
    return _NC_CACHE[tloc]


def run(inputs, trace=False):
    nc = _get_nc(T // NCORES)
    in_maps = prep_in_maps(inputs)
    res = run_bass_kernel_spmd(nc, in_maps, core_ids=list(range(NCORES)), trace=trace)
    out = np.concatenate(
        [res.results[c]["outT"].T for c in range(NCORES)], axis=0
    ).astype(np.float32)
    return np.ascontiguousarray(out), res


def kernel(**inputs):
    out, _ = run(inputs)
    return out
